# revision 10
# baseline (speedup 1.0000x reference)
"""GCN (3-layer, PyG GCNConv semantics) on 8 Trainium2 NeuronCores.

Strategy:
  - Nodes dst-sharded across 8 cores (12544-row padded chunks).
  - Per layer, activation table t = dis * h (bf16) is AllGathered so each
    core can gather any source row locally; deg^-1/2 factors are folded into
    table pre-scale and output post-scale, so edge messages need no per-edge
    math at all.
  - Edge aggregation: dma_gather (4 SWDGE queues) pulls source rows
    token-major into SBUF; segment-sums are one-hot bf16 matmuls on the PE
    accumulating per-128-dst-window PSUM tiles. No scatter is used.
  - GCNConv is computed aggregate-first ((A_sym h) W); the node-major agg
    result is bounced through HBM with a bf16 DMA-transpose to obtain the
    feature-major operand the PE needs for the dense W matmul.
  - The Bass program is jitted to this particular graph: all edge structure
    is baked into idx/segment inputs; the instruction schedule is uniform
    across cores (per-superblock/quarter run lengths are maxed over cores).
"""

import math
import numpy as np
import ml_dtypes

NEG = 0.01


# ---------------------------------------------------------------- planner --
class Cfg:
    def __init__(self, N, E, G, IN, H, OUT, NCORES=8):
        self.N, self.E, self.G, self.IN, self.H, self.OUT = N, E, G, IN, H, OUT
        self.NC = NCORES
        self.L = N // NCORES                      # real rows per core
        self.LP = ((self.L + 127) // 128) * 128   # padded rows per core
        self.NTAB = self.LP * NCORES              # AG'd table rows
        self.Q = self.NTAB // 4                   # quarter size (int16 safe)
        assert self.Q <= 32767
        self.NW = self.LP // 128                  # 128-dst windows per core
        self.SBW = 6                              # windows per superblock
        self.NSB = (self.NW + self.SBW - 1) // self.SBW
        self.GPN = N // G                         # nodes per graph


def _wrap16(idx):
    # idx [T] int -> [128, T/16] int16 (i at [i%16, i//16], replicated x8)
    a = idx.reshape(-1, 16).T
    return np.tile(a, (8, 1)).astype(np.int16).copy()


def build_plan(cfg, edge_index, batch):
    """Host-side structure planning. Returns (plan, per-core data dicts)."""
    src = np.asarray(edge_index[0], np.int64)
    dst = np.asarray(edge_index[1], np.int64)
    N, NC, L, LP, Q = cfg.N, cfg.NC, cfg.L, cfg.LP, cfg.Q

    deg = np.bincount(dst, minlength=N).astype(np.float64) + 1.0
    dis = (1.0 / np.sqrt(deg)).astype(np.float32)

    grow_of = lambda n: (n // L) * LP + (n % L)   # global table row
    gsrc = grow_of(src)

    batch = np.asarray(batch, np.int64)
    mask = np.concatenate([[True], batch[1:] != batch[:-1]])
    masked_nodes = np.nonzero(mask)[0]

    cores = []
    for k in range(NC):
        sel = (dst >= k * L) & (dst < (k + 1) * L)
        dl = (dst[sel] - k * L).astype(np.int64)
        gs = gsrc[sel]
        w = dl // 128
        sb = w // cfg.SBW
        q = gs // Q
        order = np.lexsort((dl, q, sb))
        cores.append({"dl": dl[order], "gs": gs[order], "w": w[order],
                      "sb": sb[order], "q": q[order]})

    # run lengths per (sb, q): tiles, maxed over cores
    T = np.zeros((cfg.NSB, 4), np.int64)
    for k in range(NC):
        c = cores[k]
        for s in range(cfg.NSB):
            for qq in range(4):
                cnt = int(np.sum((c["sb"] == s) & (c["q"] == qq)))
                T[s, qq] = max(T[s, qq], (cnt + 127) // 128)
    ntok = int(T.sum()) * 128

    # matmul list: for each (sb,q,tile): union over cores of windows touched
    mm_list = []   # (sb, q, tile, slot)
    tok_base = {}
    base = 0
    for s in range(cfg.NSB):
        for qq in range(4):
            tok_base[(s, qq)] = base
            base += int(T[s, qq]) * 128
    for s in range(cfg.NSB):
        for qq in range(4):
            for j in range(int(T[s, qq])):
                slots = set()
                for k in range(NC):
                    c = cores[k]
                    m = (c["sb"] == s) & (c["q"] == qq)
                    wloc = c["w"][m]
                    lo, hi = j * 128, (j + 1) * 128
                    ww = wloc[lo:hi] if lo < wloc.shape[0] else wloc[0:0]
                    slots |= set((ww % cfg.SBW).tolist())
                for sl in sorted(slots):
                    mm_list.append((s, qq, j, sl))
    # start/stop flags per window in issue order
    first_of, last_of = {}, {}
    for i, (s, qq, j, sl) in enumerate(mm_list):
        key = (s, sl)
        if key not in first_of:
            first_of[key] = i
        last_of[key] = i
    flags = [(i == first_of[(s, sl)], i == last_of[(s, sl)])
             for i, (s, qq, j, sl) in enumerate(mm_list)]

    # gather calls: slices of each (sb,q) run, <=7 tiles each
    calls = []   # (tok_start, ntiles, quarter)
    for s in range(cfg.NSB):
        for qq in range(4):
            t = int(T[s, qq])
            j = 0
            while j < t:
                n = min(7, t - j)
                calls.append((tok_base[(s, qq)] + j * 128, n, qq))
                j += n

    # per-core gather idx + segment one-hots
    nmm = len(mm_list)
    per_core = []
    for k in range(NC):
        c = cores[k]
        gidx = np.zeros(ntok, np.int64)
        seg = np.zeros((nmm, 128, 128), np.uint8)
        tok_of = {}
        for s in range(cfg.NSB):
            for qq in range(4):
                m = (c["sb"] == s) & (c["q"] == qq)
                gs = c["gs"][m]
                dl = c["dl"][m]
                b = tok_base[(s, qq)]
                gidx[b:b + gs.shape[0]] = gs - qq * Q
                tok_of[(s, qq)] = (gs.shape[0], dl)
        for i, (s, qq, j, sl) in enumerate(mm_list):
            cnt, dl = tok_of[(s, qq)]
            lo, hi = j * 128, min((j + 1) * 128, cnt)
            if lo >= hi:
                continue
            dd = dl[lo:hi]
            w_here = dd // 128
            want = (w_here % cfg.SBW == sl) & (w_here // cfg.SBW == s)
            rows = np.nonzero(want)[0] + (lo - j * 128)
            cols = dd[want] - (s * cfg.SBW + sl) * 128
            seg[i, rows, cols] = 1
        per_core.append({"gidx": gidx, "seg": seg})

    # ---- layer-3 mini-plan (masked dsts only, self-loops as tokens) ----
    m_nodes_per_core = [masked_nodes[(masked_nodes >= k * L) &
                                     (masked_nodes < (k + 1) * L)]
                        for k in range(NC)]
    MK = max(len(m) for m in m_nodes_per_core)
    assert MK <= 16
    T3 = np.zeros(4, np.int64)
    l3 = []
    for k in range(NC):
        mn = m_nodes_per_core[k]
        slot_of = {int(n): i for i, n in enumerate(mn)}
        sel = np.isin(dst, mn)
        e_s = gsrc[sel]
        e_d = dst[sel]
        # self tokens
        s_s = grow_of(mn)
        s_d = mn
        as_ = np.concatenate([e_s, s_s])
        ad = np.concatenate([e_d, s_d])
        qs = as_ // Q
        order = np.lexsort((ad, qs))
        as_, ad, qs = as_[order], ad[order], qs[order]
        l3.append({"gs": as_, "d": ad, "q": qs, "slot_of": slot_of})
        for qq in range(4):
            cnt = int(np.sum(qs == qq))
            T3[qq] = max(T3[qq], (cnt + 127) // 128)
    ntok3 = int(T3.sum()) * 128
    base3 = np.concatenate([[0], np.cumsum(T3 * 128)])[:4]
    calls3 = []
    for qq in range(4):
        j = 0
        while j < int(T3[qq]):
            n = min(7, int(T3[qq]) - j)
            calls3.append((int(base3[qq]) + j * 128, n, qq))
            j += n
    nmm3 = int(T3.sum())
    for k in range(NC):
        c = l3[k]
        gidx3 = np.zeros(ntok3, np.int64)
        seg3 = np.zeros((nmm3, 128, 16), np.uint8)
        mi = 0
        for qq in range(4):
            m = c["q"] == qq
            gs, ds = c["gs"][m], c["d"][m]
            b = int(base3[qq])
            gidx3[b:b + gs.shape[0]] = gs - qq * Q
            for j in range(int(T3[qq])):
                lo, hi = j * 128, min((j + 1) * 128, gs.shape[0])
                if lo < hi:
                    rows = np.arange(lo, hi) - j * 128
                    cols = np.array([c["slot_of"][int(d)] for d in ds[lo:hi]])
                    seg3[mi + j, rows, cols] = 1
            mi += int(T3[qq])
        per_core[k]["gidx3"] = gidx3
        per_core[k]["seg3"] = seg3
        per_core[k]["mcount"] = len(m_nodes_per_core[k])

    mm3_flags = [(i == 0, i == nmm3 - 1) for i in range(nmm3)]
    plan = {"T": T, "ntok": ntok, "mm": mm_list, "flags": flags,
            "calls": calls, "nmm": nmm, "tok_base": tok_base,
            "T3": T3, "ntok3": ntok3, "calls3": calls3, "nmm3": nmm3,
            "mm3_flags": mm3_flags, "MK": MK,
            "dis": dis, "masked_per_core": m_nodes_per_core}
    return plan, per_core


# ---------------------------------------------------------------- builder --
def build_bass(cfg, plan):
    import concourse.bacc as bacc
    import concourse.bass as bass
    import concourse.mybir as mybir
    from concourse.tile import TileContext
    from concourse import dve_ops
    from concourse.dve_spec import Spec, Src0, Src1, C0, C2, maxx, lower
    from concourse.dve_uop import DveOpSpec

    # ---- register custom fused epilogue DVE ops (idempotent) ----
    from concourse.dve_spec import _has_src1 as has_src1

    def _mkop(name, spec):
        for op in dve_ops.OPS:
            if op.name == name:
                return op
        opcode = dve_ops._CUSTOM_DVE_ROW_BASE + len(dve_ops.OPS)
        dve_ops._SUB_OPCODE_FOR_NAME[name] = opcode
        uops_sha = {}
        for ver in ("v3", "v4"):
            try:
                sp = DveOpSpec(name=name, opcode=opcode,
                               uops=lower(spec, ver=ver),
                               rd1_en=has_src1(spec))
                uops_sha[ver] = sp.sha(ver)
            except Exception:
                pass
        op = dve_ops.DveOp(name, spec, subdim=False, uops_sha=uops_sha)
        dve_ops.OPS.append(op)
        dve_ops.CUSTOM_DVE_SPECS[name] = spec
        return op

    OPU = _mkop("GCN_AGG_SCALE", Spec(
        body=(Src0 + Src1) * C0,
        reference=lambda in0, in1, s0, s1, imm2: (
            (in0.astype(np.float32) + in1.astype(np.float32)) * s0),
    ))
    OPT = _mkop("GCN_LEAKY_SCALE", Spec(
        body=maxx(Src0 + Src1, (Src0 + Src1) * C2) * C0,
        reference=lambda in0, in1, s0, s1, imm2: (
            np.maximum(in0 + in1, (in0 + in1) * imm2) * s0),
    ))

    f32, bf16, i16, u8 = (mybir.dt.float32, mybir.dt.bfloat16,
                          mybir.dt.int16, mybir.dt.uint8)
    IN, H, OUT, LP, NTAB, Q = cfg.IN, cfg.H, cfg.OUT, cfg.LP, cfg.NTAB, cfg.Q
    NW, NT = cfg.NW, LP // 128
    ntok, nmm = plan["ntok"], plan["nmm"]
    ntok3, nmm3 = plan["ntok3"], plan["nmm3"]
    MK = 16

    nc = bacc.Bacc("TRN2", target_bir_lowering=False, debug=False,
                   num_devices=cfg.NC, num_swdge_queues=4)

    xin = nc.dram_tensor("x", [LP, IN], f32, kind="ExternalInput")
    disin = nc.dram_tensor("dis", [128, NT], f32, kind="ExternalInput")
    dismin = nc.dram_tensor("dism", [MK, 1], f32, kind="ExternalInput")
    w1in = nc.dram_tensor("w1", [IN, H], bf16, kind="ExternalInput")
    w2in = nc.dram_tensor("w2", [H, H], bf16, kind="ExternalInput")
    w3in = nc.dram_tensor("w3", [H, OUT], bf16, kind="ExternalInput")
    b1in = nc.dram_tensor("b1r", [128, H], f32, kind="ExternalInput")
    b2in = nc.dram_tensor("b2r", [128, H], f32, kind="ExternalInput")
    b3in = nc.dram_tensor("b3", [MK, 1], f32, kind="ExternalInput")
    gidxin = nc.dram_tensor("gidx", [128, ntok // 16], i16, kind="ExternalInput")
    segin = nc.dram_tensor("seg", [128, nmm * 128], bf16, kind="ExternalInput")
    gidx3in = nc.dram_tensor("gidx3", [128, max(ntok3 // 16, 16)], i16,
                             kind="ExternalInput")
    seg3in = nc.dram_tensor("seg3", [128, max(nmm3 * 16, 16)], bf16,
                            kind="ExternalInput")
    outt = nc.dram_tensor("out", [MK, MK], f32, kind="ExternalOutput")

    # internal DRAM
    tb = [nc.dram_tensor(f"t{l}b", [LP, F], bf16)
          for l, F in ((0, IN), (1, H), (2, H))]
    TT = [nc.dram_tensor(f"T{l}", [NTAB, F], bf16, addr_space="Shared")
          for l, F in ((0, IN), (1, H), (2, H))]
    UU = [nc.dram_tensor("U1", [LP, IN], bf16),
          nc.dram_tensor("U2", [LP, H], bf16)]
    U3 = nc.dram_tensor("U3", [MK, H], bf16)

    rg = [list(range(cfg.NC))]

    with TileContext(nc) as tc:
        with (
            tc.tile_pool(name="const", bufs=1) as constp,
            tc.tile_pool(name="tloc", bufs=1) as tlocp,
            tc.tile_pool(name="ut", bufs=1) as utp,
            tc.tile_pool(name="msg", bufs=5) as msgp,
            tc.tile_pool(name="segt", bufs=4) as segp,
            tc.tile_pool(name="small", bufs=4) as smallp,
            tc.tile_pool(name="psA", bufs=6, space="PSUM") as psA,
            tc.tile_pool(name="psB", bufs=1, space="PSUM") as psB,
        ):
            dis_t = constp.tile([128, NT], f32)
            nc.sync.dma_start(out=dis_t[:, :], in_=disin[:, :])
            dism_t = constp.tile([MK, 1], f32)
            nc.sync.dma_start(out=dism_t[:, :], in_=dismin[:, :])
            b3_t = constp.tile([MK, 1], f32)
            nc.sync.dma_start(out=b3_t[:, :], in_=b3in[:, :])
            w1_t = constp.tile([IN, H], bf16)
            nc.sync.dma_start(out=w1_t[:, :], in_=w1in[:, :])
            w2_t = constp.tile([128, 2 * H], bf16)
            nc.sync.dma_start(
                out=w2_t[:, :].rearrange("p (ks f) -> p ks f", ks=2),
                in_=w2in.ap().rearrange("(ks p) f -> p ks f", p=128))
            w3_t = constp.tile([128, 2 * OUT], bf16)
            nc.sync.dma_start(
                out=w3_t[:, :].rearrange("p (ks f) -> p ks f", ks=2),
                in_=w3in.ap().rearrange("(ks p) f -> p ks f", p=128))
            b1_t = constp.tile([128, H], f32)
            nc.sync.dma_start(out=b1_t[:, :], in_=b1in[:, :])
            b2_t = constp.tile([128, H], f32)
            nc.sync.dma_start(out=b2_t[:, :], in_=b2in[:, :])
            gidx_t = constp.tile([128, ntok // 16], i16)
            nc.sync.dma_start(out=gidx_t[:, :], in_=gidxin[:, :])
            gidx3_t = constp.tile([128, max(ntok3 // 16, 16)], i16)
            nc.sync.dma_start(out=gidx3_t[:, :], in_=gidx3in[:, :])

            # ---- t0 = dis * x ----
            t0_loc = tlocp.tile([128, NT * IN], bf16, tag="tloc")
            for t in range(NT):
                xt = smallp.tile([128, IN], f32, tag="xt")
                nc.sync.dma_start(out=xt[:, :],
                                  in_=xin.ap().rearrange("(t p) f -> t p f", p=128)[t, :, :])
                nc.vector.tensor_scalar_mul(
                    t0_loc[:, bass.ts(t, IN)], xt[:, :], dis_t[:, t:t + 1])
            nc.sync.dma_start(
                out=tb[0].ap().rearrange("(t p) f -> p t f", p=128),
                in_=t0_loc[:, :].rearrange("p (t f) -> p t f", f=IN))
            nc.gpsimd.collective_compute(
                "AllGather", mybir.AluOpType.bypass, replica_groups=rg,
                ins=[tb[0].ap().opt()], outs=[TT[0].ap().opt()])

            callctr = [0]

            def agg_layer(lidx, F, t_loc_cur, u_dram):
                """aggregate table lidx -> u (=dis*(sum+self)) in u_dram"""
                Ttab = TT[lidx]
                cw = {}
                for ci, (tok0, ntiles, qq) in enumerate(plan["calls"]):
                    msg = msgp.tile([128, 7, F], bf16, tag="msg")
                    g = nc.gpsimd.dma_gather(
                        msg[:, 0:ntiles, :],
                        Ttab[Q * qq:Q * (qq + 1), :],
                        gidx_t[:, tok0 // 16:(tok0 + ntiles * 128) // 16],
                        ntiles * 128, ntiles * 128, F,
                        single_packet=False, queue_num=callctr[0] % 4)
                    callctr[0] += 1
                    for j in range(ntiles):
                        cw[tok0 // 128 + j] = (msg, j)
                psum_of = {}
                for i, (s, qq, j, sl) in enumerate(plan["mm"]):
                    st, sp = plan["flags"][i]
                    w = s * cfg.SBW + sl
                    if w >= NW:
                        continue
                    if st or w not in psum_of:
                        psum_of[w] = psA.tile([128, F], f32, tag="aggps", name=f"aggps_{lidx}_{w}")
                    seg_t = segp.tile([128, 128], bf16, tag="seg")
                    nc.sync.dma_start(
                        out=seg_t[:, :], in_=segin[:, bass.ts(i, 128)])
                    gtile = plan["tok_base"][(s, qq)] // 128 + j
                    msg, jj = cw[gtile]
                    nc.tensor.matmul(psum_of[w][:, :], seg_t[:, :],
                                     msg[:, jj, :], start=st, stop=sp)
                    if sp:
                        ut = smallp.tile([128, F], bf16, tag="uo")
                        nc.vector._custom_dve(
                            OPU, out=ut[:, :], in0=psum_of[w][:, :],
                            in1=t_loc_cur[:, w * F:(w + 1) * F],
                            s0=dis_t[:, w:w + 1], s1=0.0, imm2=0.0)
                        nc.sync.dma_start(
                            out=u_dram.ap().rearrange(
                                "(t p) f -> t p f", p=128)[w, :, :],
                            in_=ut[:, :])

            def dense_layer(F_in, F_out, u_dram, wt, bias_t, t_next, tb_next,
                            T_next):
                uT = utp.tile([128, (F_in // 128) * LP], bf16, tag="ut")
                for fs in range(F_in // 128):
                    nc.sync.dma_start(
                        out=uT[:, fs * LP:(fs + 1) * LP],
                        in_=u_dram[:, bass.ts(fs, 128)], transpose=True)
                for t in range(NT):
                    ps = psB.tile([128, F_out], f32, tag="wps")
                    for ks in range(F_in // 128):
                        nc.tensor.matmul(
                            ps[:, :],
                            uT[:, ks * LP + t * 128: ks * LP + (t + 1) * 128],
                            wt[:, ks * F_out:(ks + 1) * F_out],
                            start=(ks == 0), stop=(ks == F_in // 128 - 1))
                    nc.vector._custom_dve(
                        OPT, out=t_next[:, t * F_out:(t + 1) * F_out],
                        in0=ps[:, :], in1=bias_t[:, :],
                        s0=dis_t[:, t:t + 1], s1=0.0, imm2=NEG)
                nc.sync.dma_start(
                    out=tb_next.ap().rearrange("(t p) f -> p t f", p=128),
                    in_=t_next[:, :].rearrange("p (t f) -> p t f", f=F_out))
                inst = nc.gpsimd.collective_compute(
                    "AllGather", mybir.AluOpType.bypass, replica_groups=rg,
                    ins=[tb_next.ap().opt()], outs=[T_next.ap().opt()])
                return inst

            # ---- layer 1 ----
            agg_layer(0, IN, t0_loc, UU[0])
            t1_loc = tlocp.tile([128, NT * H], bf16, tag="tloc2")
            dense_layer(IN, H, UU[0], w1_t, b1_t, t1_loc, tb[1], TT[1])

            # ---- layer 2 ----
            def agg_layer2():
                Ttab = TT[1]
                F = H
                cw = {}
                for ci, (tok0, ntiles, qq) in enumerate(plan["calls"]):
                    msg = msgp.tile([128, 7, F], bf16, tag="msg")
                    g = nc.gpsimd.dma_gather(
                        msg[:, 0:ntiles, :],
                        Ttab[Q * qq:Q * (qq + 1), :],
                        gidx_t[:, tok0 // 16:(tok0 + ntiles * 128) // 16],
                        ntiles * 128, ntiles * 128, F,
                        single_packet=False, queue_num=callctr[0] % 4)
                    callctr[0] += 1
                    for j in range(ntiles):
                        cw[tok0 // 128 + j] = (msg, j)
                psum_of = {}
                for i, (s, qq, j, sl) in enumerate(plan["mm"]):
                    st, sp = plan["flags"][i]
                    w = s * cfg.SBW + sl
                    if w >= NW:
                        continue
                    if st or w not in psum_of:
                        psum_of[w] = psA.tile([128, F], f32, tag="aggps", name=f"aggps2_{w}")
                    seg_t = segp.tile([128, 128], bf16, tag="seg")
                    nc.sync.dma_start(
                        out=seg_t[:, :], in_=segin[:, bass.ts(i, 128)])
                    gtile = plan["tok_base"][(s, qq)] // 128 + j
                    msg, jj = cw[gtile]
                    nc.tensor.matmul(psum_of[w][:, :], seg_t[:, :],
                                     msg[:, jj, :], start=st, stop=sp)
                    if sp:
                        ut = smallp.tile([128, F], bf16, tag="uo")
                        nc.vector._custom_dve(
                            OPU, out=ut[:, :], in0=psum_of[w][:, :],
                            in1=t1_loc[:, w * F:(w + 1) * F],
                            s0=dis_t[:, w:w + 1], s1=0.0, imm2=0.0)
                        nc.sync.dma_start(
                            out=UU[1].ap().rearrange(
                                "(t p) f -> t p f", p=128)[w, :, :],
                            in_=ut[:, :])

            agg_layer2()
            t2_loc = tlocp.tile([128, NT * H], bf16, tag="tloc")
            dense_layer(H, H, UU[1], w2_t, b2_t, t2_loc, tb[2], TT[2])

            # ---- layer 3 (masked dsts only) ----
            ps3 = psB.tile([MK, H], f32, tag="wps", name="ps3")
            mm3i = 0
            cw3 = {}
            for ci, (tok0, ntiles, qq) in enumerate(plan["calls3"]):
                msg = msgp.tile([128, 7, H], bf16, tag="msg")
                g = nc.gpsimd.dma_gather(
                    msg[:, 0:ntiles, :],
                    TT[2][Q * qq:Q * (qq + 1), :],
                    gidx3_t[:, tok0 // 16:(tok0 + ntiles * 128) // 16],
                    ntiles * 128, ntiles * 128, H,
                    single_packet=False, queue_num=callctr[0] % 4)
                callctr[0] += 1
                for j in range(ntiles):
                    cw3[tok0 // 128 + j] = (msg, j)
            for i in range(plan["nmm3"]):
                st, sp = plan["mm3_flags"][i]
                seg_t = segp.tile([128, 16], bf16, tag="seg3")
                nc.sync.dma_start(out=seg_t[:, :],
                                    in_=seg3in[:, bass.ts(i, 16)])
                msg, jj = cw3[i]
                nc.tensor.matmul(ps3[:, :], seg_t[:, :], msg[:, jj, :],
                                 start=st, stop=sp)
            u3t = smallp.tile([MK, H], bf16, tag="u3")
            nc.vector.tensor_scalar_mul(u3t[:, :], ps3[:, :], dism_t[:, :])
            nc.sync.dma_start(out=U3[:, :], in_=u3t[:, :])
            u3T = smallp.tile([128, 2 * MK], bf16, tag="u3T")
            for fs in range(2):
                nc.sync.dma_start(out=u3T[:, fs * MK:(fs + 1) * MK],
                                  in_=U3[:, bass.ts(fs, 128)], transpose=True)
            ps4 = psB.tile([OUT, MK], f32, tag="ps4")
            for ks in range(2):
                nc.tensor.matmul(ps4[:, :],
                                 w3_t[:, ks * OUT:(ks + 1) * OUT],
                                 u3T[:, ks * MK:(ks + 1) * MK],
                                 start=(ks == 0), stop=(ks == 1))
            ot = smallp.tile([OUT, MK], f32, tag="ot")
            nc.vector.tensor_scalar_add(ot[:, :], ps4[:, :], b3_t[0:OUT, :])
            nc.sync.dma_start(out=outt[0:OUT, :], in_=ot[:, :])

    nc.finalize()
    return nc


# ----------------------------------------------------------------- driver --
def _make_inputs(cfg, plan, per_core, x, W1, b1, W2, b2, W3, b3):
    bf = ml_dtypes.bfloat16
    NT = cfg.LP // 128
    dis = plan["dis"]
    in_maps = []
    for k in range(cfg.NC):
        lo, hi = k * cfg.L, (k + 1) * cfg.L
        xk = np.zeros((cfg.LP, cfg.IN), np.float32)
        xk[:cfg.L] = x[lo:hi]
        disk = np.zeros((cfg.LP,), np.float32)
        disk[:cfg.L] = dis[lo:hi]
        dis_t = disk.reshape(NT, 128).T.copy()
        mn = plan["masked_per_core"][k]
        dism = np.zeros((16, 1), np.float32)
        dism[:len(mn), 0] = dis[mn]
        pc = per_core[k]
        seg = np.ascontiguousarray(
            pc["seg"].transpose(1, 0, 2).reshape(128, -1)).astype(bf)
        seg3 = np.ascontiguousarray(
            pc["seg3"].transpose(1, 0, 2).reshape(128, -1)).astype(bf)
        if seg3.shape[1] < 16:
            seg3 = np.zeros((128, 16), bf)
        g3 = pc["gidx3"]
        if g3.shape[0] < 256:
            g3 = np.zeros(256, np.int64)
        in_maps.append({
            "x": xk, "dis": dis_t, "dism": dism,
            "w1": W1.astype(bf), "w2": W2.astype(bf), "w3": W3.astype(bf),
            "b1r": np.tile(b1[None, :], (128, 1)).astype(np.float32),
            "b2r": np.tile(b2[None, :], (128, 1)).astype(np.float32),
            "b3": np.pad(b3, (0, 16 - cfg.OUT)).reshape(16, 1).astype(np.float32),
            "gidx": _wrap16(pc["gidx"]), "seg": seg,
            "gidx3": _wrap16(g3), "seg3": seg3,
        })
    return in_maps


def _assemble(cfg, plan, results):
    outs = []
    for k in range(cfg.NC):
        o = results[k]["out"]       # [16, 16] = [feat, node]
        m = len(plan["masked_per_core"][k])
        outs.append(o[:cfg.OUT, :m].T)
    return np.concatenate(outs, 0).astype(np.float32)


def kernel(x, edge_index, batch, W1, b1, W2, b2, W3, b3):
    from concourse.bass_utils import run_bass_kernel_spmd
    x = np.asarray(x)
    cfg = Cfg(N=x.shape[0], E=np.asarray(edge_index).shape[1],
              G=int(np.asarray(batch).max()) + 1,
              IN=x.shape[1], H=np.asarray(W2).shape[0],
              OUT=np.asarray(W3).shape[1])
    plan, per_core = build_plan(cfg, np.asarray(edge_index), np.asarray(batch))
    nc = build_bass(cfg, plan)
    in_maps = _make_inputs(cfg, plan, per_core, x,
                           np.asarray(W1), np.asarray(b1),
                           np.asarray(W2), np.asarray(b2),
                           np.asarray(W3), np.asarray(b3))
    res = run_bass_kernel_spmd(nc, in_maps, list(range(cfg.NC)))
    return _assemble(cfg, plan, res.results)


# revision 12
# speedup vs baseline: 1.8582x; 1.8582x over previous
"""GCN (3-layer, PyG GCNConv semantics) on 8 Trainium2 NeuronCores.

Strategy:
  - Nodes dst-sharded across 8 cores (12544-row padded chunks).
  - Per layer, activation table t = dis * h (bf16) is AllGathered so each
    core can gather any source row locally; deg^-1/2 factors are folded into
    table pre-scale and output post-scale, so edge messages need no per-edge
    math at all.
  - Edge aggregation: dma_gather (4 SWDGE queues) pulls source rows
    token-major into SBUF; segment-sums are one-hot bf16 matmuls on the PE
    accumulating per-128-dst-window PSUM tiles. No scatter is used.
  - GCNConv is computed aggregate-first ((A_sym h) W); the node-major agg
    result is bounced through HBM with a bf16 DMA-transpose to obtain the
    feature-major operand the PE needs for the dense W matmul.
  - The Bass program is jitted to this particular graph: all edge structure
    is baked into idx/segment inputs; the instruction schedule is uniform
    across cores (per-superblock/quarter run lengths are maxed over cores).
"""

import math
import numpy as np
import ml_dtypes

NEG = 0.01


# ---------------------------------------------------------------- planner --
class Cfg:
    def __init__(self, N, E, G, IN, H, OUT, NCORES=8):
        self.N, self.E, self.G, self.IN, self.H, self.OUT = N, E, G, IN, H, OUT
        self.NC = NCORES
        self.L = N // NCORES                      # real rows per core
        self.LP = ((self.L + 127) // 128) * 128   # padded rows per core
        self.NTAB = self.LP * NCORES              # AG'd table rows
        self.Q = self.NTAB // 4                   # quarter size (int16 safe)
        assert self.Q <= 32767
        self.NW = self.LP // 128                  # 128-dst windows per core
        self.SBW = 6                              # windows per superblock
        self.NSB = (self.NW + self.SBW - 1) // self.SBW
        self.GPN = N // G                         # nodes per graph


def _wrap16(idx):
    # idx [T] int -> [128, T/16] int16 (i at [i%16, i//16], replicated x8)
    a = idx.reshape(-1, 16).T
    return np.tile(a, (8, 1)).astype(np.int16).copy()


def build_plan(cfg, edge_index, batch):
    """Host-side structure planning. Returns (plan, per-core data dicts)."""
    src = np.asarray(edge_index[0], np.int64)
    dst = np.asarray(edge_index[1], np.int64)
    N, NC, L, LP, Q = cfg.N, cfg.NC, cfg.L, cfg.LP, cfg.Q

    deg = np.bincount(dst, minlength=N).astype(np.float64) + 1.0
    dis = (1.0 / np.sqrt(deg)).astype(np.float32)

    grow_of = lambda n: (n // L) * LP + (n % L)   # global table row
    gsrc = grow_of(src)

    batch = np.asarray(batch, np.int64)
    mask = np.concatenate([[True], batch[1:] != batch[:-1]])
    masked_nodes = np.nonzero(mask)[0]

    cores = []
    for k in range(NC):
        sel = (dst >= k * L) & (dst < (k + 1) * L)
        dl = (dst[sel] - k * L).astype(np.int64)
        gs = gsrc[sel]
        w = dl // 128
        sb = w // cfg.SBW
        q = gs // Q
        order = np.lexsort((dl, q, sb))
        cores.append({"dl": dl[order], "gs": gs[order], "w": w[order],
                      "sb": sb[order], "q": q[order]})

    # run lengths per (sb, q): tiles, maxed over cores
    T = np.zeros((cfg.NSB, 4), np.int64)
    for k in range(NC):
        c = cores[k]
        for s in range(cfg.NSB):
            for qq in range(4):
                cnt = int(np.sum((c["sb"] == s) & (c["q"] == qq)))
                T[s, qq] = max(T[s, qq], (cnt + 127) // 128)
    ntok = int(T.sum()) * 128

    # matmul list: for each (sb,q,tile): union over cores of windows touched
    mm_list = []   # (sb, q, tile, slot)
    tok_base = {}
    base = 0
    for s in range(cfg.NSB):
        for qq in range(4):
            tok_base[(s, qq)] = base
            base += int(T[s, qq]) * 128
    for s in range(cfg.NSB):
        for qq in range(4):
            for j in range(int(T[s, qq])):
                slots = set()
                for k in range(NC):
                    c = cores[k]
                    m = (c["sb"] == s) & (c["q"] == qq)
                    wloc = c["w"][m]
                    lo, hi = j * 128, (j + 1) * 128
                    ww = wloc[lo:hi] if lo < wloc.shape[0] else wloc[0:0]
                    slots |= set((ww % cfg.SBW).tolist())
                for sl in sorted(slots):
                    mm_list.append((s, qq, j, sl))
    # start/stop flags per window in issue order
    first_of, last_of = {}, {}
    for i, (s, qq, j, sl) in enumerate(mm_list):
        key = (s, sl)
        if key not in first_of:
            first_of[key] = i
        last_of[key] = i
    flags = [(i == first_of[(s, sl)], i == last_of[(s, sl)])
             for i, (s, qq, j, sl) in enumerate(mm_list)]

    # contiguous matmul index ranges per (sb, q) for batched seg loads
    mm_range = {}
    for i, (ss, qq, j, sl) in enumerate(mm_list):
        key = (ss, qq)
        lo, hi = mm_range.get(key, (i, i))
        mm_range[key] = (min(lo, i), max(hi, i + 1))

    # gather calls: slices of each (sb,q) run, <=7 tiles each
    calls = []   # (tok_start, ntiles, quarter)
    for s in range(cfg.NSB):
        for qq in range(4):
            t = int(T[s, qq])
            j = 0
            while j < t:
                n = min(7, t - j)
                calls.append((tok_base[(s, qq)] + j * 128, n, qq))
                j += n

    # per-core gather idx + segment one-hots
    nmm = len(mm_list)
    per_core = []
    for k in range(NC):
        c = cores[k]
        gidx = np.zeros(ntok, np.int64)
        seg = np.zeros((nmm, 128, 128), np.uint8)
        tok_of = {}
        for s in range(cfg.NSB):
            for qq in range(4):
                m = (c["sb"] == s) & (c["q"] == qq)
                gs = c["gs"][m]
                dl = c["dl"][m]
                b = tok_base[(s, qq)]
                gidx[b:b + gs.shape[0]] = gs - qq * Q
                tok_of[(s, qq)] = (gs.shape[0], dl)
        for i, (s, qq, j, sl) in enumerate(mm_list):
            cnt, dl = tok_of[(s, qq)]
            lo, hi = j * 128, min((j + 1) * 128, cnt)
            if lo >= hi:
                continue
            dd = dl[lo:hi]
            w_here = dd // 128
            want = (w_here % cfg.SBW == sl) & (w_here // cfg.SBW == s)
            rows = np.nonzero(want)[0] + (lo - j * 128)
            cols = dd[want] - (s * cfg.SBW + sl) * 128
            seg[i, rows, cols] = 1
        per_core.append({"gidx": gidx, "seg": seg})

    # ---- layer-3 mini-plan (masked dsts only, self-loops as tokens) ----
    m_nodes_per_core = [masked_nodes[(masked_nodes >= k * L) &
                                     (masked_nodes < (k + 1) * L)]
                        for k in range(NC)]
    MK = max(len(m) for m in m_nodes_per_core)
    assert MK <= 16
    T3 = np.zeros(4, np.int64)
    l3 = []
    for k in range(NC):
        mn = m_nodes_per_core[k]
        slot_of = {int(n): i for i, n in enumerate(mn)}
        sel = np.isin(dst, mn)
        e_s = gsrc[sel]
        e_d = dst[sel]
        # self tokens
        s_s = grow_of(mn)
        s_d = mn
        as_ = np.concatenate([e_s, s_s])
        ad = np.concatenate([e_d, s_d])
        qs = as_ // Q
        order = np.lexsort((ad, qs))
        as_, ad, qs = as_[order], ad[order], qs[order]
        l3.append({"gs": as_, "d": ad, "q": qs, "slot_of": slot_of})
        for qq in range(4):
            cnt = int(np.sum(qs == qq))
            T3[qq] = max(T3[qq], (cnt + 127) // 128)
    ntok3 = int(T3.sum()) * 128
    base3 = np.concatenate([[0], np.cumsum(T3 * 128)])[:4]
    calls3 = []
    for qq in range(4):
        j = 0
        while j < int(T3[qq]):
            n = min(7, int(T3[qq]) - j)
            calls3.append((int(base3[qq]) + j * 128, n, qq))
            j += n
    nmm3 = int(T3.sum())
    for k in range(NC):
        c = l3[k]
        gidx3 = np.zeros(ntok3, np.int64)
        seg3 = np.zeros((nmm3, 128, 16), np.uint8)
        mi = 0
        for qq in range(4):
            m = c["q"] == qq
            gs, ds = c["gs"][m], c["d"][m]
            b = int(base3[qq])
            gidx3[b:b + gs.shape[0]] = gs - qq * Q
            for j in range(int(T3[qq])):
                lo, hi = j * 128, min((j + 1) * 128, gs.shape[0])
                if lo < hi:
                    rows = np.arange(lo, hi) - j * 128
                    cols = np.array([c["slot_of"][int(d)] for d in ds[lo:hi]])
                    seg3[mi + j, rows, cols] = 1
            mi += int(T3[qq])
        per_core[k]["gidx3"] = gidx3
        per_core[k]["seg3"] = seg3
        per_core[k]["mcount"] = len(m_nodes_per_core[k])

    mm3_flags = [(i == 0, i == nmm3 - 1) for i in range(nmm3)]
    plan = {"T": T, "ntok": ntok, "mm": mm_list, "flags": flags,
            "calls": calls, "nmm": nmm, "tok_base": tok_base,
            "mm_range": mm_range,
            "T3": T3, "ntok3": ntok3, "calls3": calls3, "nmm3": nmm3,
            "mm3_flags": mm3_flags, "MK": MK,
            "dis": dis, "masked_per_core": m_nodes_per_core}
    return plan, per_core


# ---------------------------------------------------------------- builder --
def build_bass(cfg, plan):
    import concourse.bacc as bacc
    import concourse.bass as bass
    import concourse.mybir as mybir
    from concourse.tile import TileContext
    from concourse import dve_ops
    from concourse.dve_spec import Spec, Src0, Src1, C0, C2, maxx, lower
    from concourse.dve_uop import DveOpSpec

    # ---- register custom fused epilogue DVE ops (idempotent) ----
    from concourse.dve_spec import _has_src1 as has_src1

    def _mkop(name, spec):
        for op in dve_ops.OPS:
            if op.name == name:
                return op
        opcode = dve_ops._CUSTOM_DVE_ROW_BASE + len(dve_ops.OPS)
        dve_ops._SUB_OPCODE_FOR_NAME[name] = opcode
        uops_sha = {}
        for ver in ("v3", "v4"):
            try:
                sp = DveOpSpec(name=name, opcode=opcode,
                               uops=lower(spec, ver=ver),
                               rd1_en=has_src1(spec))
                uops_sha[ver] = sp.sha(ver)
            except Exception:
                pass
        op = dve_ops.DveOp(name, spec, subdim=False, uops_sha=uops_sha)
        dve_ops.OPS.append(op)
        dve_ops.CUSTOM_DVE_SPECS[name] = spec
        return op

    OPU = _mkop("GCN_AGG_SCALE", Spec(
        body=(Src0 + Src1) * C0,
        reference=lambda in0, in1, s0, s1, imm2: (
            (in0.astype(np.float32) + in1.astype(np.float32)) * s0),
    ))
    OPT = _mkop("GCN_LEAKY_SCALE", Spec(
        body=maxx(Src0 + Src1, (Src0 + Src1) * C2) * C0,
        reference=lambda in0, in1, s0, s1, imm2: (
            np.maximum(in0 + in1, (in0 + in1) * imm2) * s0),
    ))

    f32, bf16, i16, u8 = (mybir.dt.float32, mybir.dt.bfloat16,
                          mybir.dt.int16, mybir.dt.uint8)
    IN, H, OUT, LP, NTAB, Q = cfg.IN, cfg.H, cfg.OUT, cfg.LP, cfg.NTAB, cfg.Q
    NW, NT = cfg.NW, LP // 128
    ntok, nmm = plan["ntok"], plan["nmm"]
    ntok3, nmm3 = plan["ntok3"], plan["nmm3"]
    MK = 16

    nc = bacc.Bacc("TRN2", target_bir_lowering=False, debug=False,
                   num_devices=cfg.NC, num_swdge_queues=4)

    xin = nc.dram_tensor("x", [LP, IN], f32, kind="ExternalInput")
    disin = nc.dram_tensor("dis", [128, NT], f32, kind="ExternalInput")
    dismin = nc.dram_tensor("dism", [MK, 1], f32, kind="ExternalInput")
    w1in = nc.dram_tensor("w1", [IN, H], bf16, kind="ExternalInput")
    w2in = nc.dram_tensor("w2", [H, H], bf16, kind="ExternalInput")
    w3in = nc.dram_tensor("w3", [H, OUT], bf16, kind="ExternalInput")
    b1in = nc.dram_tensor("b1r", [128, H], f32, kind="ExternalInput")
    b2in = nc.dram_tensor("b2r", [128, H], f32, kind="ExternalInput")
    b3in = nc.dram_tensor("b3", [MK, 1], f32, kind="ExternalInput")
    gidxin = nc.dram_tensor("gidx", [128, ntok // 16], i16, kind="ExternalInput")
    segin = nc.dram_tensor("seg", [128, nmm * 128], bf16, kind="ExternalInput")
    gidx3in = nc.dram_tensor("gidx3", [128, max(ntok3 // 16, 16)], i16,
                             kind="ExternalInput")
    seg3in = nc.dram_tensor("seg3", [128, max(nmm3 * 16, 16)], bf16,
                            kind="ExternalInput")
    outt = nc.dram_tensor("out", [MK, MK], f32, kind="ExternalOutput")

    # internal DRAM
    tb = [nc.dram_tensor(f"t{l}b", [LP, F], bf16)
          for l, F in ((0, IN), (1, H), (2, H))]
    TT = [nc.dram_tensor(f"T{l}", [NTAB, F], bf16, addr_space="Shared")
          for l, F in ((0, IN), (1, H), (2, H))]
    UU = [nc.dram_tensor("U1", [LP, IN], bf16),
          nc.dram_tensor("U2", [LP, H], bf16)]
    U3 = nc.dram_tensor("U3", [MK, H], bf16)

    rg = [list(range(cfg.NC))]

    with TileContext(nc) as tc:
        with (
            tc.tile_pool(name="const", bufs=1) as constp,
            tc.tile_pool(name="ut", bufs=1) as utp,
            tc.tile_pool(name="msg", bufs=6) as msgp,
            tc.tile_pool(name="segt", bufs=3) as segp,
            tc.tile_pool(name="small", bufs=4) as smallp,
            tc.tile_pool(name="psA", bufs=6, space="PSUM") as psA,
            tc.tile_pool(name="psB", bufs=1, space="PSUM") as psB,
        ):
            dis_t = constp.tile([128, NT], f32)
            nc.sync.dma_start(out=dis_t[:, :], in_=disin[:, :])
            dism_t = constp.tile([MK, 1], f32)
            nc.sync.dma_start(out=dism_t[:, :], in_=dismin[:, :])
            b3_t = constp.tile([MK, 1], f32)
            nc.sync.dma_start(out=b3_t[:, :], in_=b3in[:, :])
            w1_t = constp.tile([IN, H], bf16)
            nc.sync.dma_start(out=w1_t[:, :], in_=w1in[:, :])
            w2_t = constp.tile([128, 2 * H], bf16)
            nc.sync.dma_start(
                out=w2_t[:, :].rearrange("p (ks f) -> p ks f", ks=2),
                in_=w2in.ap().rearrange("(ks p) f -> p ks f", p=128))
            w3_t = constp.tile([128, 2 * OUT], bf16)
            nc.sync.dma_start(
                out=w3_t[:, :].rearrange("p (ks f) -> p ks f", ks=2),
                in_=w3in.ap().rearrange("(ks p) f -> p ks f", p=128))
            b1_t = constp.tile([128, H], f32)
            nc.sync.dma_start(out=b1_t[:, :], in_=b1in[:, :])
            b2_t = constp.tile([128, H], f32)
            nc.sync.dma_start(out=b2_t[:, :], in_=b2in[:, :])
            gidx_t = constp.tile([128, ntok // 16], i16)
            nc.sync.dma_start(out=gidx_t[:, :], in_=gidxin[:, :])
            gidx3_t = constp.tile([128, max(ntok3 // 16, 16)], i16)
            nc.sync.dma_start(out=gidx3_t[:, :], in_=gidx3in[:, :])

            # ---- t0 = dis * x ----
            for t in range(NT):
                xt = smallp.tile([128, IN], f32, tag="xt")
                nc.sync.dma_start(out=xt[:, :],
                                  in_=xin.ap().rearrange("(t p) f -> t p f", p=128)[t, :, :])
                t0t = smallp.tile([128, IN], bf16, tag="t0t")
                nc.vector.tensor_scalar_mul(
                    t0t[:, :], xt[:, :], dis_t[:, t:t + 1])
                nc.sync.dma_start(
                    out=tb[0].ap().rearrange("(t p) f -> t p f", p=128)[t, :, :],
                    in_=t0t[:, :])
            nc.gpsimd.collective_compute(
                "AllGather", mybir.AluOpType.bypass, replica_groups=rg,
                ins=[tb[0].ap().opt()], outs=[TT[0].ap().opt()])

            callctr = [0]

            def agg_layer(lidx, F, u_dram):
                """aggregate table lidx -> u (=dis*(sum+self)) in u_dram"""
                Ttab = TT[lidx]
                tbl = tb[lidx]
                cw = {}
                for ci, (tok0, ntiles, qq) in enumerate(plan["calls"]):
                    msg = msgp.tile([128, 7, F], bf16, tag="msg",
                                    name=f"msg_{lidx}_{ci}")
                    nc.gpsimd.dma_gather(
                        msg[:, 0:ntiles, :],
                        Ttab[Q * qq:Q * (qq + 1), :],
                        gidx_t[:, tok0 // 16:(tok0 + ntiles * 128) // 16],
                        ntiles * 128, ntiles * 128, F,
                        single_packet=False, queue_num=callctr[0] % 4)
                    callctr[0] += 1
                    for j in range(ntiles):
                        cw[tok0 // 128 + j] = (msg, j)
                seg_ar = {}
                psum_of = {}
                for i, (s, qq, j, sl) in enumerate(plan["mm"]):
                    st, sp = plan["flags"][i]
                    w = s * cfg.SBW + sl
                    if w >= NW:
                        continue
                    key = (s, qq)
                    if key not in seg_ar:
                        lo, hi = plan["mm_range"][key]
                        ar = segp.tile([128, (hi - lo) * 128], bf16,
                                       tag="seg", name=f"seg_{lidx}_{s}_{qq}")
                        nc.sync.dma_start(
                            out=ar[:, :],
                            in_=segin[:, lo * 128:hi * 128])
                        seg_ar[key] = (ar, lo)
                    ar, lo = seg_ar[key]
                    if st or w not in psum_of:
                        psum_of[w] = psA.tile([128, F], f32, tag="aggps", name=f"aggps_{lidx}_{w}")
                    gtile = plan["tok_base"][(s, qq)] // 128 + j
                    msg, jj = cw[gtile]
                    nc.tensor.matmul(psum_of[w][:, :],
                                     ar[:, bass.ts(i - lo, 128)],
                                     msg[:, jj, :], start=st, stop=sp)
                    if sp:
                        tl = smallp.tile([128, F], bf16, tag="tl",
                                         name=f"tl_{lidx}_{w}")
                        nc.sync.dma_start(
                            out=tl[:, :],
                            in_=tbl.ap().rearrange(
                                "(t p) f -> t p f", p=128)[w, :, :])
                        ut = smallp.tile([128, F], bf16, tag="uo",
                                         name=f"uo_{lidx}_{w}")
                        nc.vector._custom_dve(
                            OPU, out=ut[:, :], in0=psum_of[w][:, :],
                            in1=tl[:, :],
                            s0=dis_t[:, w:w + 1], s1=0.0, imm2=0.0)
                        nc.sync.dma_start(
                            out=u_dram.ap().rearrange(
                                "(t p) f -> t p f", p=128)[w, :, :],
                            in_=ut[:, :])

            def dense_layer(F_in, F_out, u_dram, wt, bias_t, tb_next,
                            T_next):
                uT = utp.tile([128, (F_in // 128) * LP], bf16, tag="ut")
                for fs in range(F_in // 128):
                    nc.sync.dma_start(
                        out=uT[:, fs * LP:(fs + 1) * LP],
                        in_=u_dram[:, bass.ts(fs, 128)], transpose=True)
                for t in range(NT):
                    ps = psB.tile([128, F_out], f32, tag="wps")
                    for ks in range(F_in // 128):
                        nc.tensor.matmul(
                            ps[:, :],
                            uT[:, ks * LP + t * 128: ks * LP + (t + 1) * 128],
                            wt[:, ks * F_out:(ks + 1) * F_out],
                            start=(ks == 0), stop=(ks == F_in // 128 - 1))
                    tn = smallp.tile([128, F_out], bf16, tag="tl",
                                     name=f"tn_{id(tb_next)}_{t}")
                    nc.vector._custom_dve(
                        OPT, out=tn[:, :],
                        in0=ps[:, :], in1=bias_t[:, :],
                        s0=dis_t[:, t:t + 1], s1=0.0, imm2=NEG)
                    nc.sync.dma_start(
                        out=tb_next.ap().rearrange(
                            "(t p) f -> t p f", p=128)[t, :, :],
                        in_=tn[:, :])
                inst = nc.gpsimd.collective_compute(
                    "AllGather", mybir.AluOpType.bypass, replica_groups=rg,
                    ins=[tb_next.ap().opt()], outs=[T_next.ap().opt()])
                return inst

            # ---- layer 1 ----
            agg_layer(0, IN, UU[0])
            dense_layer(IN, H, UU[0], w1_t, b1_t, tb[1], TT[1])

            # ---- layer 2 ----
            agg_layer(1, H, UU[1])
            dense_layer(H, H, UU[1], w2_t, b2_t, tb[2], TT[2])

            # ---- layer 3 (masked dsts only) ----
            ps3 = psB.tile([MK, H], f32, tag="wps", name="ps3")
            mm3i = 0
            cw3 = {}
            for ci, (tok0, ntiles, qq) in enumerate(plan["calls3"]):
                msg = msgp.tile([128, 7, H], bf16, tag="msg")
                g = nc.gpsimd.dma_gather(
                    msg[:, 0:ntiles, :],
                    TT[2][Q * qq:Q * (qq + 1), :],
                    gidx3_t[:, tok0 // 16:(tok0 + ntiles * 128) // 16],
                    ntiles * 128, ntiles * 128, H,
                    single_packet=False, queue_num=callctr[0] % 4)
                callctr[0] += 1
                for j in range(ntiles):
                    cw3[tok0 // 128 + j] = (msg, j)
            seg3_t = segp.tile([128, max(plan["nmm3"] * 16, 16)], bf16,
                               tag="seg", name="seg3all")
            nc.sync.dma_start(out=seg3_t[:, :], in_=seg3in[:, :])
            for i in range(plan["nmm3"]):
                st, sp = plan["mm3_flags"][i]
                msg, jj = cw3[i]
                nc.tensor.matmul(ps3[:, :], seg3_t[:, bass.ts(i, 16)],
                                 msg[:, jj, :], start=st, stop=sp)
            u3t = smallp.tile([MK, H], bf16, tag="u3")
            nc.vector.tensor_scalar_mul(u3t[:, :], ps3[:, :], dism_t[:, :])
            nc.sync.dma_start(out=U3[:, :], in_=u3t[:, :])
            u3T = smallp.tile([128, 2 * MK], bf16, tag="u3T")
            for fs in range(2):
                nc.sync.dma_start(out=u3T[:, fs * MK:(fs + 1) * MK],
                                  in_=U3[:, bass.ts(fs, 128)], transpose=True)
            ps4 = psB.tile([OUT, MK], f32, tag="ps4")
            for ks in range(2):
                nc.tensor.matmul(ps4[:, :],
                                 w3_t[:, ks * OUT:(ks + 1) * OUT],
                                 u3T[:, ks * MK:(ks + 1) * MK],
                                 start=(ks == 0), stop=(ks == 1))
            ot = smallp.tile([OUT, MK], f32, tag="ot")
            nc.vector.tensor_scalar_add(ot[:, :], ps4[:, :], b3_t[0:OUT, :])
            nc.sync.dma_start(out=outt[0:OUT, :], in_=ot[:, :])

    nc.finalize()
    return nc


# ----------------------------------------------------------------- driver --
def _make_inputs(cfg, plan, per_core, x, W1, b1, W2, b2, W3, b3):
    bf = ml_dtypes.bfloat16
    NT = cfg.LP // 128
    dis = plan["dis"]
    in_maps = []
    for k in range(cfg.NC):
        lo, hi = k * cfg.L, (k + 1) * cfg.L
        xk = np.zeros((cfg.LP, cfg.IN), np.float32)
        xk[:cfg.L] = x[lo:hi]
        disk = np.zeros((cfg.LP,), np.float32)
        disk[:cfg.L] = dis[lo:hi]
        dis_t = disk.reshape(NT, 128).T.copy()
        mn = plan["masked_per_core"][k]
        dism = np.zeros((16, 1), np.float32)
        dism[:len(mn), 0] = dis[mn]
        pc = per_core[k]
        seg = np.ascontiguousarray(
            pc["seg"].transpose(1, 0, 2).reshape(128, -1)).astype(bf)
        seg3 = np.ascontiguousarray(
            pc["seg3"].transpose(1, 0, 2).reshape(128, -1)).astype(bf)
        if seg3.shape[1] < 16:
            seg3 = np.zeros((128, 16), bf)
        g3 = pc["gidx3"]
        if g3.shape[0] < 256:
            g3 = np.zeros(256, np.int64)
        in_maps.append({
            "x": xk, "dis": dis_t, "dism": dism,
            "w1": W1.astype(bf), "w2": W2.astype(bf), "w3": W3.astype(bf),
            "b1r": np.tile(b1[None, :], (128, 1)).astype(np.float32),
            "b2r": np.tile(b2[None, :], (128, 1)).astype(np.float32),
            "b3": np.pad(b3, (0, 16 - cfg.OUT)).reshape(16, 1).astype(np.float32),
            "gidx": _wrap16(pc["gidx"]), "seg": seg,
            "gidx3": _wrap16(g3), "seg3": seg3,
        })
    return in_maps


def _assemble(cfg, plan, results):
    outs = []
    for k in range(cfg.NC):
        o = results[k]["out"]       # [16, 16] = [feat, node]
        m = len(plan["masked_per_core"][k])
        outs.append(o[:cfg.OUT, :m].T)
    return np.concatenate(outs, 0).astype(np.float32)


def kernel(x, edge_index, batch, W1, b1, W2, b2, W3, b3):
    from concourse.bass_utils import run_bass_kernel_spmd
    x = np.asarray(x)
    cfg = Cfg(N=x.shape[0], E=np.asarray(edge_index).shape[1],
              G=int(np.asarray(batch).max()) + 1,
              IN=x.shape[1], H=np.asarray(W2).shape[0],
              OUT=np.asarray(W3).shape[1])
    plan, per_core = build_plan(cfg, np.asarray(edge_index), np.asarray(batch))
    nc = build_bass(cfg, plan)
    in_maps = _make_inputs(cfg, plan, per_core, x,
                           np.asarray(W1), np.asarray(b1),
                           np.asarray(W2), np.asarray(b2),
                           np.asarray(W3), np.asarray(b3))
    res = run_bass_kernel_spmd(nc, in_maps, list(range(cfg.NC)))
    return _assemble(cfg, plan, res.results)


# revision 14
# speedup vs baseline: 1.9567x; 1.0530x over previous
"""GCN (3-layer, PyG GCNConv semantics) on 8 Trainium2 NeuronCores.

Strategy:
  - Nodes dst-sharded across 8 cores (12544-row padded chunks).
  - Per layer, activation table t = dis * h (bf16) is AllGathered so each
    core can gather any source row locally; deg^-1/2 factors are folded into
    table pre-scale and output post-scale, so edge messages need no per-edge
    math at all.
  - Edge aggregation: dma_gather (4 SWDGE queues) pulls source rows
    token-major into SBUF; segment-sums are one-hot bf16 matmuls on the PE
    accumulating per-128-dst-window PSUM tiles. No scatter is used.
  - GCNConv is computed aggregate-first ((A_sym h) W); the node-major agg
    result is bounced through HBM with a bf16 DMA-transpose to obtain the
    feature-major operand the PE needs for the dense W matmul.
  - The Bass program is jitted to this particular graph: all edge structure
    is baked into idx/segment inputs; the instruction schedule is uniform
    across cores (per-superblock/quarter run lengths are maxed over cores).
"""

import math
import numpy as np
import ml_dtypes

NEG = 0.01


# ---------------------------------------------------------------- planner --
class Cfg:
    def __init__(self, N, E, G, IN, H, OUT, NCORES=8):
        self.N, self.E, self.G, self.IN, self.H, self.OUT = N, E, G, IN, H, OUT
        self.NC = NCORES
        self.L = N // NCORES                      # real rows per core
        self.LP = ((self.L + 127) // 128) * 128   # padded rows per core
        self.NTAB = self.LP * NCORES              # AG'd table rows
        self.Q = self.NTAB // 4                   # quarter size (int16 safe)
        assert self.Q <= 32767
        self.NW = self.LP // 128                  # 128-dst windows per core
        self.SBW = 6                              # windows per superblock
        self.NSB = (self.NW + self.SBW - 1) // self.SBW
        self.GPN = N // G                         # nodes per graph


def _wrap16(idx):
    # idx [T] int -> [128, T/16] int16 (i at [i%16, i//16], replicated x8)
    a = idx.reshape(-1, 16).T
    return np.tile(a, (8, 1)).astype(np.int16).copy()


def build_plan(cfg, edge_index, batch):
    """Host-side structure planning. Returns (plan, per-core data dicts)."""
    src = np.asarray(edge_index[0], np.int64)
    dst = np.asarray(edge_index[1], np.int64)
    N, NC, L, LP, Q = cfg.N, cfg.NC, cfg.L, cfg.LP, cfg.Q

    deg = np.bincount(dst, minlength=N).astype(np.float64) + 1.0
    dis = (1.0 / np.sqrt(deg)).astype(np.float32)

    grow_of = lambda n: (n // L) * LP + (n % L)   # global table row
    gsrc = grow_of(src)

    batch = np.asarray(batch, np.int64)
    mask = np.concatenate([[True], batch[1:] != batch[:-1]])
    masked_nodes = np.nonzero(mask)[0]

    cores = []
    for k in range(NC):
        sel = (dst >= k * L) & (dst < (k + 1) * L)
        dl = (dst[sel] - k * L).astype(np.int64)
        gs = gsrc[sel]
        w = dl // 128
        sb = w // cfg.SBW
        q = gs // Q
        order = np.lexsort((dl, q, sb))
        cores.append({"dl": dl[order], "gs": gs[order], "w": w[order],
                      "sb": sb[order], "q": q[order]})

    # run lengths per (sb, q): tiles, maxed over cores
    T = np.zeros((cfg.NSB, 4), np.int64)
    for k in range(NC):
        c = cores[k]
        for s in range(cfg.NSB):
            for qq in range(4):
                cnt = int(np.sum((c["sb"] == s) & (c["q"] == qq)))
                T[s, qq] = max(T[s, qq], (cnt + 127) // 128)
    ntok = int(T.sum()) * 128

    # matmul list: for each (sb,q,tile): union over cores of windows touched
    mm_list = []   # (sb, q, tile, slot)
    tok_base = {}
    base = 0
    for s in range(cfg.NSB):
        for qq in range(4):
            tok_base[(s, qq)] = base
            base += int(T[s, qq]) * 128
    for s in range(cfg.NSB):
        for qq in range(4):
            for j in range(int(T[s, qq])):
                slots = set()
                for k in range(NC):
                    c = cores[k]
                    m = (c["sb"] == s) & (c["q"] == qq)
                    wloc = c["w"][m]
                    lo, hi = j * 128, (j + 1) * 128
                    ww = wloc[lo:hi] if lo < wloc.shape[0] else wloc[0:0]
                    slots |= set((ww % cfg.SBW).tolist())
                for sl in sorted(slots):
                    mm_list.append((s, qq, j, sl))
    # start/stop flags per window in issue order
    first_of, last_of = {}, {}
    for i, (s, qq, j, sl) in enumerate(mm_list):
        key = (s, sl)
        if key not in first_of:
            first_of[key] = i
        last_of[key] = i
    flags = [(i == first_of[(s, sl)], i == last_of[(s, sl)])
             for i, (s, qq, j, sl) in enumerate(mm_list)]

    # contiguous matmul index ranges per (sb, q) for batched seg loads
    mm_range = {}
    for i, (ss, qq, j, sl) in enumerate(mm_list):
        key = (ss, qq)
        lo, hi = mm_range.get(key, (i, i))
        mm_range[key] = (min(lo, i), max(hi, i + 1))

    # gather calls: slices of each (sb,q) run, <=7 tiles each
    calls = []   # (tok_start, ntiles, quarter)
    for s in range(cfg.NSB):
        for qq in range(4):
            t = int(T[s, qq])
            j = 0
            while j < t:
                n = min(7, t - j)
                calls.append((tok_base[(s, qq)] + j * 128, n, qq))
                j += n

    # per-core gather idx + segment one-hots
    nmm = len(mm_list)
    per_core = []
    for k in range(NC):
        c = cores[k]
        gidx = np.zeros(ntok, np.int64)
        seg = np.zeros((nmm, 128, 128), np.uint8)
        tok_of = {}
        for s in range(cfg.NSB):
            for qq in range(4):
                m = (c["sb"] == s) & (c["q"] == qq)
                gs = c["gs"][m]
                dl = c["dl"][m]
                b = tok_base[(s, qq)]
                gidx[b:b + gs.shape[0]] = gs - qq * Q
                tok_of[(s, qq)] = (gs.shape[0], dl)
        for i, (s, qq, j, sl) in enumerate(mm_list):
            cnt, dl = tok_of[(s, qq)]
            lo, hi = j * 128, min((j + 1) * 128, cnt)
            if lo >= hi:
                continue
            dd = dl[lo:hi]
            w_here = dd // 128
            want = (w_here % cfg.SBW == sl) & (w_here // cfg.SBW == s)
            rows = np.nonzero(want)[0] + (lo - j * 128)
            cols = dd[want] - (s * cfg.SBW + sl) * 128
            seg[i, rows, cols] = 1
        per_core.append({"gidx": gidx, "seg": seg})

    # ---- layer-3 mini-plan (masked dsts only, self-loops as tokens) ----
    m_nodes_per_core = [masked_nodes[(masked_nodes >= k * L) &
                                     (masked_nodes < (k + 1) * L)]
                        for k in range(NC)]
    MK = max(len(m) for m in m_nodes_per_core)
    assert MK <= 16
    T3 = np.zeros(4, np.int64)
    l3 = []
    for k in range(NC):
        mn = m_nodes_per_core[k]
        slot_of = {int(n): i for i, n in enumerate(mn)}
        sel = np.isin(dst, mn)
        e_s = gsrc[sel]
        e_d = dst[sel]
        # self tokens
        s_s = grow_of(mn)
        s_d = mn
        as_ = np.concatenate([e_s, s_s])
        ad = np.concatenate([e_d, s_d])
        qs = as_ // Q
        order = np.lexsort((ad, qs))
        as_, ad, qs = as_[order], ad[order], qs[order]
        l3.append({"gs": as_, "d": ad, "q": qs, "slot_of": slot_of})
        for qq in range(4):
            cnt = int(np.sum(qs == qq))
            T3[qq] = max(T3[qq], (cnt + 127) // 128)
    ntok3 = int(T3.sum()) * 128
    base3 = np.concatenate([[0], np.cumsum(T3 * 128)])[:4]
    calls3 = []
    for qq in range(4):
        j = 0
        while j < int(T3[qq]):
            n = min(7, int(T3[qq]) - j)
            calls3.append((int(base3[qq]) + j * 128, n, qq))
            j += n
    nmm3 = int(T3.sum())
    for k in range(NC):
        c = l3[k]
        gidx3 = np.zeros(ntok3, np.int64)
        seg3 = np.zeros((nmm3, 128, 16), np.uint8)
        mi = 0
        for qq in range(4):
            m = c["q"] == qq
            gs, ds = c["gs"][m], c["d"][m]
            b = int(base3[qq])
            gidx3[b:b + gs.shape[0]] = gs - qq * Q
            for j in range(int(T3[qq])):
                lo, hi = j * 128, min((j + 1) * 128, gs.shape[0])
                if lo < hi:
                    rows = np.arange(lo, hi) - j * 128
                    cols = np.array([c["slot_of"][int(d)] for d in ds[lo:hi]])
                    seg3[mi + j, rows, cols] = 1
            mi += int(T3[qq])
        per_core[k]["gidx3"] = gidx3
        per_core[k]["seg3"] = seg3
        per_core[k]["mcount"] = len(m_nodes_per_core[k])

    mm3_flags = [(i == 0, i == nmm3 - 1) for i in range(nmm3)]
    plan = {"T": T, "ntok": ntok, "mm": mm_list, "flags": flags,
            "calls": calls, "nmm": nmm, "tok_base": tok_base,
            "mm_range": mm_range,
            "T3": T3, "ntok3": ntok3, "calls3": calls3, "nmm3": nmm3,
            "mm3_flags": mm3_flags, "MK": MK,
            "dis": dis, "masked_per_core": m_nodes_per_core}
    return plan, per_core


# ---------------------------------------------------------------- builder --
def build_bass(cfg, plan):
    import concourse.bacc as bacc
    import concourse.bass as bass
    import concourse.mybir as mybir
    from concourse.tile import TileContext
    from concourse import dve_ops
    from concourse.dve_spec import Spec, Src0, Src1, C0, C2, maxx, lower
    from concourse.dve_uop import DveOpSpec

    # ---- register custom fused epilogue DVE ops (idempotent) ----
    from concourse.dve_spec import _has_src1 as has_src1

    def _mkop(name, spec):
        for op in dve_ops.OPS:
            if op.name == name:
                return op
        opcode = dve_ops._CUSTOM_DVE_ROW_BASE + len(dve_ops.OPS)
        dve_ops._SUB_OPCODE_FOR_NAME[name] = opcode
        uops_sha = {}
        for ver in ("v3", "v4"):
            try:
                sp = DveOpSpec(name=name, opcode=opcode,
                               uops=lower(spec, ver=ver),
                               rd1_en=has_src1(spec))
                uops_sha[ver] = sp.sha(ver)
            except Exception:
                pass
        op = dve_ops.DveOp(name, spec, subdim=False, uops_sha=uops_sha)
        dve_ops.OPS.append(op)
        dve_ops.CUSTOM_DVE_SPECS[name] = spec
        return op

    OPU = _mkop("GCN_AGG_SCALE", Spec(
        body=(Src0 + Src1) * C0,
        reference=lambda in0, in1, s0, s1, imm2: (
            (in0.astype(np.float32) + in1.astype(np.float32)) * s0),
    ))
    OPT = _mkop("GCN_LEAKY_SCALE", Spec(
        body=maxx(Src0 + Src1, (Src0 + Src1) * C2) * C0,
        reference=lambda in0, in1, s0, s1, imm2: (
            np.maximum(in0 + in1, (in0 + in1) * imm2) * s0),
    ))

    f32, bf16, i16, u8 = (mybir.dt.float32, mybir.dt.bfloat16,
                          mybir.dt.int16, mybir.dt.uint8)
    IN, H, OUT, LP, NTAB, Q = cfg.IN, cfg.H, cfg.OUT, cfg.LP, cfg.NTAB, cfg.Q
    NW, NT = cfg.NW, LP // 128
    ntok, nmm = plan["ntok"], plan["nmm"]
    ntok3, nmm3 = plan["ntok3"], plan["nmm3"]
    MK = 16

    nc = bacc.Bacc("TRN2", target_bir_lowering=False, debug=False,
                   num_devices=cfg.NC, num_swdge_queues=4)

    xin = nc.dram_tensor("x", [LP, IN], f32, kind="ExternalInput")
    disin = nc.dram_tensor("dis", [128, NT], f32, kind="ExternalInput")
    dismin = nc.dram_tensor("dism", [MK, 1], f32, kind="ExternalInput")
    w1in = nc.dram_tensor("w1", [IN, H], bf16, kind="ExternalInput")
    w2in = nc.dram_tensor("w2", [H, H], bf16, kind="ExternalInput")
    w3in = nc.dram_tensor("w3", [H, OUT], bf16, kind="ExternalInput")
    b1in = nc.dram_tensor("b1r", [128, H], f32, kind="ExternalInput")
    b2in = nc.dram_tensor("b2r", [128, H], f32, kind="ExternalInput")
    b3in = nc.dram_tensor("b3", [MK, 1], f32, kind="ExternalInput")
    gidxin = nc.dram_tensor("gidx", [128, ntok // 16], i16, kind="ExternalInput")
    segin = nc.dram_tensor("seg", [128, nmm * 128], bf16, kind="ExternalInput")
    gidx3in = nc.dram_tensor("gidx3", [128, max(ntok3 // 16, 16)], i16,
                             kind="ExternalInput")
    seg3in = nc.dram_tensor("seg3", [128, max(nmm3 * 16, 16)], bf16,
                            kind="ExternalInput")
    outt = nc.dram_tensor("out", [MK, MK], f32, kind="ExternalOutput")

    # internal DRAM
    tb = [nc.dram_tensor(f"t{l}b", [LP, F], bf16)
          for l, F in ((0, IN), (1, H), (2, H))]
    TT = [nc.dram_tensor(f"T{l}", [NTAB, F], bf16, addr_space="Shared")
          for l, F in ((0, IN), (1, H), (2, H))]
    UU = [nc.dram_tensor("U1", [LP, IN], bf16),
          nc.dram_tensor("U2", [LP, H], bf16)]
    U3 = nc.dram_tensor("U3", [MK, H], bf16)

    rg = [list(range(cfg.NC))]

    with TileContext(nc) as tc:
        with (
            tc.tile_pool(name="const", bufs=1) as constp,
            tc.tile_pool(name="ut", bufs=1) as utp,
            tc.tile_pool(name="msg", bufs=12) as msgp,
            tc.tile_pool(name="segt", bufs=4) as segp,
            tc.tile_pool(name="small", bufs=4) as smallp,
            tc.tile_pool(name="psA", bufs=6, space="PSUM") as psA,
            tc.tile_pool(name="psB", bufs=1, space="PSUM") as psB,
        ):
            dis_t = constp.tile([128, NT], f32)
            nc.sync.dma_start(out=dis_t[:, :], in_=disin[:, :])
            dism_t = constp.tile([MK, 1], f32)
            nc.sync.dma_start(out=dism_t[:, :], in_=dismin[:, :])
            b3_t = constp.tile([MK, 1], f32)
            nc.sync.dma_start(out=b3_t[:, :], in_=b3in[:, :])
            w1_t = constp.tile([IN, H], bf16)
            nc.sync.dma_start(out=w1_t[:, :], in_=w1in[:, :])
            w2_t = constp.tile([128, 2 * H], bf16)
            nc.sync.dma_start(
                out=w2_t[:, :].rearrange("p (ks f) -> p ks f", ks=2),
                in_=w2in.ap().rearrange("(ks p) f -> p ks f", p=128))
            w3_t = constp.tile([128, 2 * OUT], bf16)
            nc.sync.dma_start(
                out=w3_t[:, :].rearrange("p (ks f) -> p ks f", ks=2),
                in_=w3in.ap().rearrange("(ks p) f -> p ks f", p=128))
            b1_t = constp.tile([128, H], f32)
            nc.sync.dma_start(out=b1_t[:, :], in_=b1in[:, :])
            b2_t = constp.tile([128, H], f32)
            nc.sync.dma_start(out=b2_t[:, :], in_=b2in[:, :])
            gidx_t = constp.tile([128, ntok // 16], i16)
            nc.sync.dma_start(out=gidx_t[:, :], in_=gidxin[:, :])
            gidx3_t = constp.tile([128, max(ntok3 // 16, 16)], i16)
            nc.sync.dma_start(out=gidx3_t[:, :], in_=gidx3in[:, :])

            # ---- t0 = dis * x ----
            xa = utp.tile([128, NT * IN], f32, tag="ut", name="xarena")
            nc.sync.dma_start(
                out=xa[:, :].rearrange("p (t f) -> p t f", f=IN),
                in_=xin.ap().rearrange("(t p) f -> p t f", p=128))
            t0a = msgp.tile([128, NT * IN], bf16, tag="t0a", name="t0arena", bufs=1)
            for t in range(NT):
                nc.vector.tensor_scalar_mul(
                    t0a[:, bass.ts(t, IN)], xa[:, bass.ts(t, IN)],
                    dis_t[:, t:t + 1])
            nc.sync.dma_start(
                out=tb[0].ap().rearrange("(t p) f -> p t f", p=128),
                in_=t0a[:, :].rearrange("p (t f) -> p t f", f=IN))
            nc.gpsimd.collective_compute(
                "AllGather", mybir.AluOpType.bypass, replica_groups=rg,
                ins=[tb[0].ap().opt()], outs=[TT[0].ap().opt()])

            callctr = [0]

            def agg_layer(lidx, F, u_dram):
                """aggregate table lidx -> u (=dis*(sum+self)) in u_dram"""
                Ttab = TT[lidx]
                tbl = tb[lidx]
                cw = {}
                for ci, (tok0, ntiles, qq) in enumerate(plan["calls"]):
                    msg = msgp.tile([128, 7, F], bf16, tag="msg",
                                    name=f"msg_{lidx}_{ci}")
                    nc.gpsimd.dma_gather(
                        msg[:, 0:ntiles, :],
                        Ttab[Q * qq:Q * (qq + 1), :],
                        gidx_t[:, tok0 // 16:(tok0 + ntiles * 128) // 16],
                        ntiles * 128, ntiles * 128, F,
                        single_packet=False, queue_num=callctr[0] % 4)
                    callctr[0] += 1
                    for j in range(ntiles):
                        cw[tok0 // 128 + j] = (msg, j)
                seg_ar = {}
                psum_of = {}
                for i, (s, qq, j, sl) in enumerate(plan["mm"]):
                    st, sp = plan["flags"][i]
                    w = s * cfg.SBW + sl
                    if w >= NW:
                        continue
                    key = (s, qq)
                    if key not in seg_ar:
                        lo, hi = plan["mm_range"][key]
                        ar = segp.tile([128, (hi - lo) * 128], bf16,
                                       tag="seg", name=f"seg_{lidx}_{s}_{qq}")
                        nc.sync.dma_start(
                            out=ar[:, :],
                            in_=segin[:, lo * 128:hi * 128])
                        seg_ar[key] = (ar, lo)
                    ar, lo = seg_ar[key]
                    if st or w not in psum_of:
                        psum_of[w] = psA.tile([128, F], f32, tag="aggps", name=f"aggps_{lidx}_{w}")
                    gtile = plan["tok_base"][(s, qq)] // 128 + j
                    msg, jj = cw[gtile]
                    nc.tensor.matmul(psum_of[w][:, :],
                                     ar[:, bass.ts(i - lo, 128)],
                                     msg[:, jj, :], start=st, stop=sp)
                    if sp:
                        tl = smallp.tile([128, F], bf16, tag="tl",
                                         name=f"tl_{lidx}_{w}")
                        nc.sync.dma_start(
                            out=tl[:, :],
                            in_=tbl.ap().rearrange(
                                "(t p) f -> t p f", p=128)[w, :, :])
                        ut = smallp.tile([128, F], bf16, tag="uo",
                                         name=f"uo_{lidx}_{w}")
                        nc.vector._custom_dve(
                            OPU, out=ut[:, :], in0=psum_of[w][:, :],
                            in1=tl[:, :],
                            s0=dis_t[:, w:w + 1], s1=0.0, imm2=0.0)
                        nc.sync.dma_start(
                            out=u_dram.ap().rearrange(
                                "(t p) f -> t p f", p=128)[w, :, :],
                            in_=ut[:, :])

            def dense_layer(F_in, F_out, u_dram, wt, bias_t, tb_next,
                            T_next):
                uT = utp.tile([128, (F_in // 128) * LP], bf16, tag="ut")
                for fs in range(F_in // 128):
                    nc.sync.dma_start(
                        out=uT[:, fs * LP:(fs + 1) * LP],
                        in_=u_dram[:, bass.ts(fs, 128)], transpose=True)
                for t in range(NT):
                    ps = psB.tile([128, F_out], f32, tag="wps")
                    for ks in range(F_in // 128):
                        nc.tensor.matmul(
                            ps[:, :],
                            uT[:, ks * LP + t * 128: ks * LP + (t + 1) * 128],
                            wt[:, ks * F_out:(ks + 1) * F_out],
                            start=(ks == 0), stop=(ks == F_in // 128 - 1))
                    tn = smallp.tile([128, F_out], bf16, tag="tl",
                                     name=f"tn_{id(tb_next)}_{t}")
                    nc.vector._custom_dve(
                        OPT, out=tn[:, :],
                        in0=ps[:, :], in1=bias_t[:, :],
                        s0=dis_t[:, t:t + 1], s1=0.0, imm2=NEG)
                    nc.sync.dma_start(
                        out=tb_next.ap().rearrange(
                            "(t p) f -> t p f", p=128)[t, :, :],
                        in_=tn[:, :])
                inst = nc.gpsimd.collective_compute(
                    "AllGather", mybir.AluOpType.bypass, replica_groups=rg,
                    ins=[tb_next.ap().opt()], outs=[T_next.ap().opt()])
                return inst

            # ---- layer 1 ----
            agg_layer(0, IN, UU[0])
            dense_layer(IN, H, UU[0], w1_t, b1_t, tb[1], TT[1])

            # ---- layer 2 ----
            agg_layer(1, H, UU[1])
            dense_layer(H, H, UU[1], w2_t, b2_t, tb[2], TT[2])

            # ---- layer 3 (masked dsts only) ----
            ps3 = psB.tile([MK, H], f32, tag="wps", name="ps3")
            mm3i = 0
            cw3 = {}
            for ci, (tok0, ntiles, qq) in enumerate(plan["calls3"]):
                msg = msgp.tile([128, 7, H], bf16, tag="msg")
                g = nc.gpsimd.dma_gather(
                    msg[:, 0:ntiles, :],
                    TT[2][Q * qq:Q * (qq + 1), :],
                    gidx3_t[:, tok0 // 16:(tok0 + ntiles * 128) // 16],
                    ntiles * 128, ntiles * 128, H,
                    single_packet=False, queue_num=callctr[0] % 4)
                callctr[0] += 1
                for j in range(ntiles):
                    cw3[tok0 // 128 + j] = (msg, j)
            seg3_t = segp.tile([128, max(plan["nmm3"] * 16, 16)], bf16,
                               tag="seg", name="seg3all")
            nc.sync.dma_start(out=seg3_t[:, :], in_=seg3in[:, :])
            for i in range(plan["nmm3"]):
                st, sp = plan["mm3_flags"][i]
                msg, jj = cw3[i]
                nc.tensor.matmul(ps3[:, :], seg3_t[:, bass.ts(i, 16)],
                                 msg[:, jj, :], start=st, stop=sp)
            u3t = smallp.tile([MK, H], bf16, tag="u3")
            nc.vector.tensor_scalar_mul(u3t[:, :], ps3[:, :], dism_t[:, :])
            nc.sync.dma_start(out=U3[:, :], in_=u3t[:, :])
            u3T = smallp.tile([128, 2 * MK], bf16, tag="u3T")
            for fs in range(2):
                nc.sync.dma_start(out=u3T[:, fs * MK:(fs + 1) * MK],
                                  in_=U3[:, bass.ts(fs, 128)], transpose=True)
            ps4 = psB.tile([OUT, MK], f32, tag="ps4")
            for ks in range(2):
                nc.tensor.matmul(ps4[:, :],
                                 w3_t[:, ks * OUT:(ks + 1) * OUT],
                                 u3T[:, ks * MK:(ks + 1) * MK],
                                 start=(ks == 0), stop=(ks == 1))
            ot = smallp.tile([OUT, MK], f32, tag="ot")
            nc.vector.tensor_scalar_add(ot[:, :], ps4[:, :], b3_t[0:OUT, :])
            nc.sync.dma_start(out=outt[0:OUT, :], in_=ot[:, :])

    nc.finalize()
    return nc


# ----------------------------------------------------------------- driver --
def _make_inputs(cfg, plan, per_core, x, W1, b1, W2, b2, W3, b3):
    bf = ml_dtypes.bfloat16
    NT = cfg.LP // 128
    dis = plan["dis"]
    in_maps = []
    for k in range(cfg.NC):
        lo, hi = k * cfg.L, (k + 1) * cfg.L
        xk = np.zeros((cfg.LP, cfg.IN), np.float32)
        xk[:cfg.L] = x[lo:hi]
        disk = np.zeros((cfg.LP,), np.float32)
        disk[:cfg.L] = dis[lo:hi]
        dis_t = disk.reshape(NT, 128).T.copy()
        mn = plan["masked_per_core"][k]
        dism = np.zeros((16, 1), np.float32)
        dism[:len(mn), 0] = dis[mn]
        pc = per_core[k]
        seg = np.ascontiguousarray(
            pc["seg"].transpose(1, 0, 2).reshape(128, -1)).astype(bf)
        seg3 = np.ascontiguousarray(
            pc["seg3"].transpose(1, 0, 2).reshape(128, -1)).astype(bf)
        if seg3.shape[1] < 16:
            seg3 = np.zeros((128, 16), bf)
        g3 = pc["gidx3"]
        if g3.shape[0] < 256:
            g3 = np.zeros(256, np.int64)
        in_maps.append({
            "x": xk, "dis": dis_t, "dism": dism,
            "w1": W1.astype(bf), "w2": W2.astype(bf), "w3": W3.astype(bf),
            "b1r": np.tile(b1[None, :], (128, 1)).astype(np.float32),
            "b2r": np.tile(b2[None, :], (128, 1)).astype(np.float32),
            "b3": np.pad(b3, (0, 16 - cfg.OUT)).reshape(16, 1).astype(np.float32),
            "gidx": _wrap16(pc["gidx"]), "seg": seg,
            "gidx3": _wrap16(g3), "seg3": seg3,
        })
    return in_maps


def _assemble(cfg, plan, results):
    outs = []
    for k in range(cfg.NC):
        o = results[k]["out"]       # [16, 16] = [feat, node]
        m = len(plan["masked_per_core"][k])
        outs.append(o[:cfg.OUT, :m].T)
    return np.concatenate(outs, 0).astype(np.float32)


def kernel(x, edge_index, batch, W1, b1, W2, b2, W3, b3):
    from concourse.bass_utils import run_bass_kernel_spmd
    x = np.asarray(x)
    cfg = Cfg(N=x.shape[0], E=np.asarray(edge_index).shape[1],
              G=int(np.asarray(batch).max()) + 1,
              IN=x.shape[1], H=np.asarray(W2).shape[0],
              OUT=np.asarray(W3).shape[1])
    plan, per_core = build_plan(cfg, np.asarray(edge_index), np.asarray(batch))
    nc = build_bass(cfg, plan)
    in_maps = _make_inputs(cfg, plan, per_core, x,
                           np.asarray(W1), np.asarray(b1),
                           np.asarray(W2), np.asarray(b2),
                           np.asarray(W3), np.asarray(b3))
    res = run_bass_kernel_spmd(nc, in_maps, list(range(cfg.NC)))
    return _assemble(cfg, plan, res.results)
